# revision 4
# baseline (speedup 1.0000x reference)
"""MaxSimPartition Trainium2 kernel.

scores[b,c] = mean_q max_d ( q_vectors[b,q,:] . vectors[upids[b,c],d,:] ),
then per-row top-k over the 1024 candidates. 8-core SPMD: candidates are
sharded by column (core m takes candidate slots [128m, 128m+128) of every row).

Host: replicates the reference's unique()-with--1-padding, gathers each core's
candidate doc vectors into a transposed, chunk-packed stream (contiguous 1 MiB
DMA chunks of 16 candidates: 4 row-strips x 4 slots x 128 doc-tokens).

Device (per core, static program): per chunk one DMA + four col-tiled fp32
matmuls (lhsT = Q^T slice [128,32] of the strip's row, rhs = 4 candidate
V^T tiles [128,512]) into one PSUM bank + one segmented DVE reduce_max into a
running max tile. Per group of 4 rows, a ones-block matmul turns the maxes
into means over the 32 query tokens. Output is [4,4,128] f32 per core (8 KB).

Host: assembles [16,1024] scores, masks duplicate (-1) slots to -inf, does the
reference-identical stable top-k and gathers pids.
"""

import sys

import numpy as np

for _p in ("/opt/trn_rl_repo",):
    if _p not in sys.path:
        sys.path.append(_p)

N_CORES = 8
B, QLEN, DIM = 16, 32, 128
KPIDS = 1024
PER_CORE = KPIDS // N_CORES  # 128 candidate slots per row per core
N_GROUPS = 4                 # groups of 4 rows
CHUNKS_PER_GROUP = 32        # 4 candidates per row-strip per chunk
N_CHUNKS = N_GROUPS * CHUNKS_PER_GROUP  # 128 chunks x 16 candidates = 2048

_CACHE = {}


def _program():
    """Build + compile the per-core Bass/Tile program (cached per process)."""
    if "nc" in _CACHE:
        return _CACHE["nc"]
    import concourse.bass as bass
    import concourse.tile as tile
    from concourse import bacc, mybir

    dt = mybir.dt
    nc = bacc.Bacc("TRN2", target_bir_lowering=False, debug=False)

    vt_d = nc.dram_tensor("vt", [N_CHUNKS, 128, 2048], dt.float32, kind="ExternalInput")
    qt_d = nc.dram_tensor("qt", [128, 512], dt.float32, kind="ExternalInput")
    ones_d = nc.dram_tensor("onesb", [128, 4], dt.float32, kind="ExternalInput")
    out_d = nc.dram_tensor("means", [4, 4, PER_CORE], dt.float32, kind="ExternalOutput")

    with tile.TileContext(nc) as tc:
        with (
            tc.tile_pool(name="vpool", bufs=6) as vpool,
            tc.tile_pool(name="cpool", bufs=1) as cpool,
            tc.tile_pool(name="ps", bufs=6, space=bass.MemorySpace.PSUM) as ps,
            tc.tile_pool(name="ps2", bufs=2, space=bass.MemorySpace.PSUM) as ps2,
        ):
            qt = cpool.tile([128, 512], dt.float32)
            onesb = cpool.tile([128, 4], dt.float32)
            maxt = cpool.tile([128, N_GROUPS, PER_CORE], dt.float32)
            means = cpool.tile([4, N_GROUPS, PER_CORE], dt.float32)
            nc.sync.dma_start(qt[:], qt_d[:])
            nc.sync.dma_start(onesb[:], ones_d[:])

            for g in range(N_GROUPS):
                for c in range(CHUNKS_PER_GROUP):
                    i = CHUNKS_PER_GROUP * g + c
                    vt = vpool.tile([128, 2048], dt.float32)
                    nc.sync.dma_start(vt[:], vt_d[i])
                    acc = ps.tile([128, 512], dt.float32)
                    for j in range(4):
                        b = 4 * g + j
                        nc.tensor.matmul(
                            acc[32 * j : 32 * j + 32, :],
                            qt[:, 32 * b : 32 * b + 32],
                            vt[:, 512 * j : 512 * (j + 1)],
                            tile_position=(0, 32 * j),
                        )
                    nc.vector.reduce_max(
                        maxt[:, g, 4 * c : 4 * c + 4],
                        acc[:].rearrange("p (s d) -> p s d", d=128),
                        axis=mybir.AxisListType.X,
                    )
                mps = ps2.tile([4, PER_CORE], dt.float32)
                nc.tensor.matmul(mps[:], onesb[:], maxt[:, g, :])
                nc.vector.tensor_copy(means[:, g, :], mps[:])
            nc.sync.dma_start(out_d[:], means[:])

    nc.compile()
    _CACHE["nc"] = nc
    return nc


def _unique_pids_np(p):
    """Numpy replica of reference._unique_pids (descending sort, dups -> -1)."""
    s = -np.sort(-p, axis=1)
    dup = np.concatenate(
        [np.zeros((s.shape[0], 1), dtype=bool), s[:, 1:] == s[:, :-1]], axis=1
    )
    return -np.sort(-np.where(dup, -1, s), axis=1)


def _prepare(q_vectors, vectors, pids, boundaries):
    """Host preprocessing: unique pids + per-core packed device inputs."""
    qv = np.asarray(q_vectors, dtype=np.float32)
    V = np.asarray(vectors, dtype=np.float32)
    pids = np.asarray(pids)
    boundaries = np.asarray(boundaries)
    assert qv.shape == (B, QLEN, DIM) and V.shape[1:] == (128, DIM)
    n = V.shape[0]

    p = pids.astype(np.int64) - int(boundaries[0])
    p = np.where((p < 0) | (p >= n), -1, p)
    upids = _unique_pids_np(p)                      # [16, 1024] int64
    cand = np.clip(upids, 0, None)

    # Per-doc transpose once: VT[doc, h, d] = vectors[doc, d, h]
    VT = np.ascontiguousarray(V.transpose(0, 2, 1))

    qt = np.ascontiguousarray(qv.transpose(2, 0, 1)).reshape(128, B * QLEN)
    onesb = np.zeros((128, 4), np.float32)
    for j in range(4):
        onesb[32 * j : 32 * j + 32, j] = 1.0 / 32

    in_maps = []
    for m in range(N_CORES):
        sub = cand[:, PER_CORE * m : PER_CORE * (m + 1)]          # [16, 128]
        # chunk-major candidate order: [g, c, j, t] with b = 4g+j, s = 4c+t
        idx = sub.reshape(4, 4, CHUNKS_PER_GROUP, 4).transpose(0, 2, 1, 3).reshape(-1)
        A = VT[idx]                                               # [2048, h, d]
        vt = np.ascontiguousarray(
            A.reshape(N_CHUNKS, 16, 128, 128).transpose(0, 2, 1, 3)
        ).reshape(N_CHUNKS, 128, 2048)
        in_maps.append({"vt": vt, "qt": qt, "onesb": onesb})
    return in_maps, upids, pids.dtype


def kernel(q_vectors, vectors, pids, boundaries, k):
    from concourse.bass_utils import run_bass_kernel_spmd

    k = int(np.asarray(k))
    in_maps, upids, pid_dtype = _prepare(q_vectors, vectors, pids, boundaries)
    nc = _program()
    res = run_bass_kernel_spmd(nc, in_maps, core_ids=list(range(N_CORES)))
    return _postprocess(res.results, upids, k, pid_dtype)


def _postprocess(results, upids, k, pid_dtype):
    S = np.empty((B, KPIDS), np.float32)
    for m in range(N_CORES):
        o = results[m]["means"]                                   # [j, g, s]
        S[:, PER_CORE * m : PER_CORE * (m + 1)] = o.transpose(1, 0, 2).reshape(
            B, PER_CORE
        )
    S = np.where(upids < 0, -np.inf, S)

    order = np.argsort(-S, axis=1, kind="stable")[:, :k]
    top_scores = np.take_along_axis(S, order, axis=1).astype(np.float32)
    top_pids = np.take_along_axis(upids, order, axis=1).astype(pid_dtype)
    return top_scores, top_pids
